# revision 1
# baseline (speedup 1.0000x reference)
"""Distributed causal self-attention kernel for 8 Trainium2 NeuronCores.

Problem: B=4, T=2048, C=1024, H=16 heads, D=64 head dim, fp32.
  qkv = x @ W_qkv.T + b_qkv; causal attention per head; out = attn @ W_proj.T + b_proj

Sharding (hybrid DP x TP, no on-device collectives):
  core c -> batch b = c//2 (data parallel), head group g = c%2 (8 heads each,
  tensor parallel). Each core computes a row-parallel *partial* projection
  output for its batch; the host sums the two partials per batch (the TP
  reduction) and adds b_proj. All weights are pre-transposed / pre-scaled on
  the host so the device only runs matmuls in their natural layouts:

  - xT [C, T]: x[b] transposed -> moving/stationary operand with contraction
    (C) on partitions.
  - Q^T, K^T produced in [j, T] layout (weight-stationary matmuls); the 1/8
    attention scale is folded into W_q/b_q on the host.
  - V produced in natural [T, j] layout (x-stationary matmuls), stored per
    k-tile as [ones(64) | V_0..V_7] so the attn@V stationary [ones|V_h]
    computes the softmax denominator (replicated on partitions 0-63) and the
    un-normalized output (partitions 64-127) in a single matmul.
  - Scores are computed TRANSPOSED (scores_T[t_k, t_q] = K^T.T @ Q^T) so that
    P~ = exp(scores_T) is directly the moving operand of attn@V -- no
    transposes anywhere in the attention pipeline.
  - proj consumes attnT [j, T] directly, producing outT [C, T] partials.

  All matmul operands use float32r (tf32-like, 4x faster than fp32 on the
  PE; rel err ~1e-3 end-to-end).
"""
import sys

if "/opt/trn_rl_repo" not in sys.path:
    sys.path.insert(0, "/opt/trn_rl_repo")

import ml_dtypes
import numpy as np

import concourse.bass as bass
import concourse.tile as tile
from concourse import bacc, mybir
from concourse.bass_utils import run_bass_kernel_spmd
from concourse.masks import make_upper_triangular

F32 = mybir.dt.float32
F32R = mybir.dt.float32r
BF16 = mybir.dt.bfloat16

B, T, C = 4, 2048, 1024
H, D = 16, 64
HC = 8            # heads per core
P = 128           # partitions
NCORES = 8
NT = T // P       # 16 t-tiles of 128
NTC = T // 512    # 4 t-chunks of 512
NCT = C // P      # 8 contraction tiles for qkv
JQK = 1024        # q+k columns per core
NJT = JQK // P    # 8 j-tiles (4 q, 4 k)
JV = 512          # v columns per core
NMT = C // P      # 8 proj output row tiles
NPJ = JV // P     # 4 proj contraction tiles

_compiled = None


def build():
    nc = bacc.Bacc("TRN2", target_bir_lowering=False, debug=False,
                   num_devices=NCORES)
    x_ext = nc.declare_dram_parameter("xT", [C, T], F32R, isOutput=False)
    wqkv_ext = nc.declare_dram_parameter("wqkv", [C, 3 * JV], F32R, isOutput=False)
    bqkv_ext = nc.declare_dram_parameter("bqkv", [3 * JV], F32, isOutput=False)
    wproj_ext = nc.declare_dram_parameter("wproj", [JV, C], BF16, isOutput=False)
    bproj_ext = nc.declare_dram_parameter("bproj", [C], F32, isOutput=False)
    out_ext = nc.declare_dram_parameter("out", [C, T], F32, isOutput=True)

    with tile.TileContext(nc, pool_alloc_mode="queue") as tc:
        _body(nc, tc, x_ext, wqkv_ext, bqkv_ext, wproj_ext, bproj_ext, out_ext)
    nc.compile()
    return nc


def _body(nc, tc, x_ext, wqkv_ext, bqkv_ext, wproj_ext, bproj_ext, out_ext):
    dma = nc.default_dma_engine

    from contextlib import ExitStack
    ctx = ExitStack()
    with ctx:
        singles = ctx.enter_context(tc.tile_pool(name="singles", bufs=1))
        qkt_pool = ctx.enter_context(tc.tile_pool(name="qkT", bufs=1))
        vpool = ctx.enter_context(tc.tile_pool(name="v", bufs=1))
        apool = ctx.enter_context(tc.tile_pool(name="attnT", bufs=1))
        ptpool = ctx.enter_context(tc.tile_pool(name="pt", bufs=4))
        rspool = ctx.enter_context(tc.tile_pool(name="rs", bufs=4))
        wp_pool = ctx.enter_context(tc.tile_pool(name="wp", bufs=1))
        opool = ctx.enter_context(tc.tile_pool(name="outs", bufs=2))
        psum = ctx.enter_context(tc.tile_pool(name="psum", bufs=1, space="PSUM"))
        xpool_cm = tc.tile_pool(name="x", bufs=1, side="right")
        xpool = xpool_cm.__enter__()
        wv_cm = tc.tile_pool(name="wv", bufs=1, side="right")
        wv_pool = wv_cm.__enter__()

        # ---- HAM warmup: dummy fp32 matmuls (4 cyc/row, ~1.7us each when
        # cold) bridge the initial DMA ramp so the PE clock is at 8/8 when
        # the first real matmuls issue. Reuses an rs-pool slot: no extra SBUF.
        warm = rspool.tile([P, 512], F32, tag="rs", name="warm")
        nc.vector.memset(warm[:], 1.0)
        for i in range(6):
            wps = psum.tile([P, 512], F32, tag="mm", bufs=2, name=f"warm{i}")
            nc.tensor.matmul(wps[:], warm[:, 0:P], warm[:])

        # ---- constants ----
        mask = singles.tile([P, P], BF16)       # upper-tri (t_q >= t_k) 0/1
        make_upper_triangular(nc, mask[:], val=1.0, diag=True)

        bqk_t = singles.tile([P, NJT], F32)     # per-partition q/k biases
        dma.dma_start(out=bqk_t[:], in_=bqkv_ext[:JQK].rearrange("(j p) -> p j", p=P))
        bv_b = singles.tile([P, JV], F32)       # v bias broadcast over partitions
        bv_src = bass.AP(tensor=bqkv_ext, offset=JQK, ap=[[0, P], [1, JV]])
        dma.dma_start(out=bv_b[:], in_=bv_src)
        bproj_t = singles.tile([P, NMT], F32)
        dma.dma_start(out=bproj_t[:], in_=bproj_ext[:].rearrange("(m p) -> p m", p=P))

        # ---- x loads (column-chunked so the v pass can start early) ----
        wv = []
        for ct in range(NCT):
            wt = wv_pool.tile([P, JV], F32R, tag=f"wv{ct}", name=f"wv{ct}")
            dma.dma_start(out=wt[:, 0:256],
                          in_=wqkv_ext[ct * P:(ct + 1) * P, JQK:JQK + 256])
            dma.dma_start(out=wt[:, 256:],
                          in_=wqkv_ext[ct * P:(ct + 1) * P, JQK + 256:])
            wv.append(wt)
        xts = [xpool.tile([P, T], F32R, tag=f"x{ct}", name=f"x{ct}")
               for ct in range(NCT)]
        for tcn in range(NTC):
            for ct in range(NCT):
                dma.dma_start(
                    out=xts[ct][:, tcn * 512:(tcn + 1) * 512],
                    in_=x_ext[ct * P:(ct + 1) * P, tcn * 512:(tcn + 1) * 512])

        # ---- v pass: 16 k-tiles ----
        # v_sb[kt]: [128, 1024] = per head h: [ones(64) | V_h(64)] at col 128h;
        # ones make attn@V emit the softmax denominator on partitions 0-63
        vts = []

        def v_tile(kt):
            psv = psum.tile([P, JV], F32, tag="mm", bufs=2, name=f"psv{kt}")
            for ct in range(NCT):
                nc.tensor.matmul(
                    psv[:], xts[ct][:, kt * P:(kt + 1) * P], wv[ct][:],
                    start=(ct == 0), stop=(ct == NCT - 1),
                )
            vt = vpool.tile([P, 2 * JV], BF16, tag=f"v{kt}", name=f"v{kt}")
            vt3 = vt[:].rearrange("p (h c) -> p h c", h=HC)
            nc.vector.memset(vt3[:, :, 0:64], 1.0)
            nc.vector.tensor_add(
                vt3[:, :, 64:128],
                psv[:].rearrange("p (h c) -> p h c", h=HC),
                bv_b[:].rearrange("p (h c) -> p h c", h=HC),
            )
            vts.append(vt)

        for kt in range(4):
            v_tile(kt)

        with tc.tile_pool(name="wqk", bufs=1, side="right") as wqk_pool:
            wqk = []
            for ct in range(NCT):
                wt = wqk_pool.tile([P, JQK], F32R, tag=f"wqk{ct}", name=f"wqk{ct}")
                dma.dma_start(out=wt[:], in_=wqkv_ext[ct * P:(ct + 1) * P, :JQK])
                wqk.append(wt)
            wproj = []
            for jt in range(NPJ):
                wt = wp_pool.tile([P, C], BF16, tag=f"wp{jt}", name=f"wp{jt}")
                dma.dma_start(out=wt[:], in_=wproj_ext[jt * P:(jt + 1) * P, :])
                wproj.append(wt)

            # q/k tiles rotate between even/odd pairs (2 pairs in flight)
            qkT = {}
            attnT = [apool.tile([P, T], BF16, tag=f"a{p_}", name=f"attnT{p_}")
                     for p_ in range(4)]

            def qk_tiles(p_):
                qkT[p_] = qkt_pool.tile([P, T], BF16, tag=f"q{p_ % 2}",
                                        name=f"qT{p_}")
                qkT[4 + p_] = qkt_pool.tile([P, T], BF16, tag=f"k{p_ % 2}",
                                            name=f"kT{p_}")
                for jt in (p_, 4 + p_):
                    for tcn in range(NTC):
                        ps = psum.tile([P, 512], F32, tag="mm", bufs=2,
                                       name=f"psqk{jt}_{tcn}")
                        for ct in range(NCT):
                            nc.tensor.matmul(
                                ps[:], wqk[ct][:, jt * P:(jt + 1) * P],
                                xts[ct][:, tcn * 512:(tcn + 1) * 512],
                                start=(ct == 0), stop=(ct == NCT - 1),
                            )
                        nc.vector.tensor_scalar_add(
                            out=qkT[jt][:, tcn * 512:(tcn + 1) * 512],
                            in0=ps[:], scalar1=bqk_t[:, jt:jt + 1])

            # ---- per pair: its two qk j-tiles, then its attention ----
            # The ACT exp chain of pair p overlaps the PE qk matmuls of
            # pair p+1 (lower priority, dependency-free). Pair 0's qk is
            # emitted after only 4 v tiles so its attention (and the ACT
            # pipeline) starts early; remaining v tiles fill PE gaps.
            for p_ in range(4):
                qk_tiles(p_)
                if p_ == 0:
                    for kt in range(4, NT):
                        v_tile(kt)

                qTt = qkT[p_]
                kTt = qkT[4 + p_]
                # pair 3 runs q-chunks high-to-low so the projection (which
                # consumes chunks in the same order) tails on the SHORT
                # qc=0 block instead of the 16-ktile qc=3 block
                qcs = range(NTC) if p_ < 3 else range(NTC - 1, -1, -1)
                for qc in qcs:
                    pso = [psum.tile([P, 512], F32, tag="o", bufs=2,
                                     name=f"pso{p_}_{qc}_{i}")
                           for i in range(2)]
                    nkt = 4 * (qc + 1)
                    pending = None
                    for kt in range(nkt):
                        o = max(0, kt * P - qc * 512)
                        ss = psum.tile([P, 1024], F32, tag="s", bufs=2,
                                       name=f"pss{p_}_{qc}_{kt}")
                        for h in range(2):
                            lo, hi = h * 64, (h + 1) * 64
                            nc.tensor.matmul(
                                ss[:, 512 * h + o:512 * (h + 1)],
                                kTt[lo:hi, kt * P:(kt + 1) * P],
                                qTt[lo:hi, qc * 512 + o:(qc + 1) * 512],
                            )
                        pt = ptpool.tile([P, 1024], BF16, tag="pt",
                                         name=f"pt{p_}_{qc}_{kt}")
                        ss3 = ss[:].rearrange("p (h w) -> p h w", h=2)
                        pt3 = pt[:].rearrange("p (h w) -> p h w", h=2)
                        nc.scalar.activation(
                            pt3[:, :, o:], ss3[:, :, o:],
                            mybir.ActivationFunctionType.Exp,
                        )
                        if kt >= 4 * qc:
                            # diagonal block: zero t_q < t_k (both heads)
                            mask_b = bass.AP(
                                tensor=mask[:].tensor, offset=mask[:].offset,
                                ap=[mask[:].ap[0], [0, 2], [1, P]])
                            nc.vector.tensor_mul(
                                pt3[:, :, o:o + P], pt3[:, :, o:o + P], mask_b)
                        if pending is not None:
                            _emit_av(nc, vts, pso, p_, *pending, nkt)
                        pending = (pt, o, kt)
                    _emit_av(nc, vts, pso, p_, *pending, nkt)
                    # normalize: pso rows 0:64 = row-sum, 64:128 = outT
                    for h in range(2):
                        rsb = rspool.tile([P, 512], F32, tag="rs",
                                          name=f"rs{p_}_{qc}_{h}")
                        # fast recip is lane-locked: compute at base 0 (frees
                        # the psum fast), DMA-shift to partitions 64-127
                        nc.vector.reciprocal_approx_fast(
                            rsb[0:64, :], pso[h][0:64, :])
                        dma.dma_start(out=rsb[64:128, :], in_=rsb[0:64, :])
                        lo = 64 * h
                        nc.vector.tensor_mul(
                            attnT[p_][lo:lo + 64, qc * 512:(qc + 1) * 512],
                            pso[h][64:128, :], rsb[64:128, :])

            # ---- projection: outT[m, t] partial; t-chunk outer, matching
            # pair 3's reversed qc order so each chunk unlocks asap ----
            for tcn in range(NTC - 1, -1, -1):
                for mt in range(NMT):
                    psp = psum.tile([P, 512], F32, tag="mm", bufs=2,
                                    name=f"psp{mt}_{tcn}")
                    for jt in range(NPJ):
                        nc.tensor.matmul(
                            psp[:], wproj[jt][:, mt * P:(mt + 1) * P],
                            attnT[jt][:, tcn * 512:(tcn + 1) * 512],
                            start=(jt == 0), stop=(jt == NPJ - 1),
                        )
                    ot = opool.tile([P, 512], F32, tag="ot", name=f"ot{mt}_{tcn}")
                    nc.vector.tensor_scalar_add(
                        out=ot[:], in0=psp[:], scalar1=bproj_t[:, mt:mt + 1])
                    dma.dma_start(
                        out=out_ext[mt * P:(mt + 1) * P,
                                    tcn * 512:(tcn + 1) * 512],
                        in_=ot[:])

        wv_cm.__exit__(None, None, None)
        xpool_cm.__exit__(None, None, None)


def _emit_av(nc, vts, pso, p_, pt, o, kt, nkt):
    """attn@V for one (pair, kt) block: [ones|V_h].T @ P~ accumulated."""
    for h in range(2):
        head = 2 * p_ + h
        vaug = vts[kt][:, 128 * head:128 * head + 128]
        nc.tensor.matmul(
            pso[h][:, o:], vaug, pt[:, 512 * h + o:512 * (h + 1)],
            start=(kt == 0), stop=(kt == nkt - 1),
        )


def shard_inputs(x, W_qkv, b_qkv, W_proj, b_proj):
    """Build the 8 per-core input maps (host-side sharding)."""
    x = np.asarray(x, np.float32)
    W_qkv = np.asarray(W_qkv, np.float32)
    b_qkv = np.asarray(b_qkv, np.float32)
    W_proj = np.asarray(W_proj, np.float32)
    b_proj = np.asarray(b_proj, np.float32)
    in_maps = []
    for c in range(NCORES):
        b, g = c // 2, c % 2
        s = slice(512 * g, 512 * g + 512)
        Wq = W_qkv[0 * C:1 * C][s] * 0.125
        Wk = W_qkv[1 * C:2 * C][s]
        Wv = W_qkv[2 * C:3 * C][s]
        wqkv = np.ascontiguousarray(np.concatenate([Wq, Wk, Wv], 0).T)
        bq = b_qkv[0 * C:1 * C][s] * 0.125
        bk = b_qkv[1 * C:2 * C][s]
        bv = b_qkv[2 * C:3 * C][s]
        in_maps.append({
            "xT": np.ascontiguousarray(x[b].T),
            "wqkv": wqkv,
            "bqkv": np.ascontiguousarray(np.concatenate([bq, bk, bv])),
            "wproj": np.ascontiguousarray(W_proj[:, s].T).astype(ml_dtypes.bfloat16),
            "bproj": b_proj if g == 0 else np.zeros_like(b_proj),
        })
    return in_maps


def run(in_maps, trace=False):
    global _compiled
    if _compiled is None:
        _compiled = build()
    return run_bass_kernel_spmd(
        _compiled, in_maps, core_ids=list(range(NCORES)), trace=trace)


def kernel(x, W_qkv, b_qkv, W_proj, b_proj):
    in_maps = shard_inputs(x, W_qkv, b_qkv, W_proj, b_proj)
    res = run(in_maps)
    out = np.empty((B, T, C), np.float32)
    for b in range(B):
        partial = res.results[2 * b]["out"] + res.results[2 * b + 1]["out"]
        out[b] = partial.T
    return out


if __name__ == "__main__":
    rng = np.random.default_rng(0)
    xs = {
        "x": rng.standard_normal((B, T, C)).astype(np.float32),
        "W_qkv": (rng.standard_normal((3 * C, C)) / 32).astype(np.float32),
        "b_qkv": (rng.standard_normal(3 * C) * 0.02).astype(np.float32),
        "W_proj": (rng.standard_normal((C, C)) / 32).astype(np.float32),
        "b_proj": (rng.standard_normal(C) * 0.02).astype(np.float32),
    }
    out = kernel(**xs)
    print("out", out.shape, out.dtype, np.abs(out).mean())



# revision 5
# speedup vs baseline: 1.1990x; 1.1990x over previous
"""Distributed causal self-attention kernel for 8 Trainium2 NeuronCores.

Problem: B=4, T=2048, C=1024, H=16 heads, D=64 head dim, fp32.
  qkv = x @ W_qkv.T + b_qkv; causal attention per head; out = attn @ W_proj.T + b_proj

Sharding (hybrid DP x TP, no on-device collectives):
  core c -> batch b = c//2 (data parallel), head group g = c%2 (8 heads each,
  tensor parallel). Each core computes a row-parallel *partial* projection
  output for its batch; the host sums the two partials per batch (the TP
  reduction) and adds b_proj.

v2 vs baseline (353.7us):
  - all device tensors bf16 (x, W, outputs): halves DMA, same PE rate.
  - per-chunk tiles (x, qkT, attnT split at 512-column granularity) so the
    Tile scheduler's range deps never serialize whole phases.
  - emission order = scheduler priority: attention blocks are emitted as
    early as their data deps allow; v-pass remainder / next pair's qk /
    projection chunks are emitted AFTER so they fill PE slack instead of
    starving the ACT exp chain (the baseline's exp chain started at ~99us;
    target ~15us).
  - dummy EXP at t=0 pre-loads the ACT spline table (~2.7us) off-path.
  - wqkv columns regrouped pair-major ([q0|k0|q1|k1|...|v]) so pair 0's
    weights arrive in one early DMA.
  - b_proj folded on the host (it must sum TP partials anyway); outputs bf16.
  - one combined reciprocal-shift DMA per (pair, qc) instead of two.

  All matmuls bf16 (1 cyc/row): scores computed TRANSPOSED (no transposes
  anywhere); AV stationary [ones|V_h] emits softmax denominator + raw output
  in one matmul; 1/8 scale folded into W_q/b_q host-side.
"""
import sys

if "/opt/trn_rl_repo" not in sys.path:
    sys.path.insert(0, "/opt/trn_rl_repo")

import ml_dtypes
import numpy as np

import concourse.bass as bass
import concourse.tile as tile
from concourse import bacc, mybir
from concourse.bass_utils import run_bass_kernel_spmd
from concourse.masks import make_upper_triangular

F32 = mybir.dt.float32
BF16 = mybir.dt.bfloat16

B, T, C = 4, 2048, 1024
H, D = 16, 64
HC = 8            # heads per core
P = 128           # partitions
NCORES = 8
NT = T // P       # 16 t-tiles of 128
NTC = T // 512    # 4 t-chunks of 512
NCT = C // P      # 8 contraction tiles for qkv
JQK = 1024        # q+k columns per core
JV = 512          # v columns per core
NMT = C // P      # 8 proj output row tiles
NPJ = JV // P     # 4 proj contraction tiles

_compiled = None


def build():
    nc = bacc.Bacc("TRN2", target_bir_lowering=False, debug=False,
                   num_devices=NCORES)
    x_ext = nc.declare_dram_parameter("xT", [C, T], BF16, isOutput=False)
    # columns pair-major: [q0|k0|q1|k1|q2|k2|q3|k3 | v]
    wqkv_ext = nc.declare_dram_parameter("wqkv", [C, 3 * JV], BF16, isOutput=False)
    bqkv_ext = nc.declare_dram_parameter("bqkv", [3 * JV], F32, isOutput=False)
    wproj_ext = nc.declare_dram_parameter("wproj", [JV, C], BF16, isOutput=False)
    out_ext = nc.declare_dram_parameter("out", [C, T], BF16, isOutput=True)

    with tile.TileContext(nc, pool_alloc_mode="queue") as tc:
        _body(nc, tc, x_ext, wqkv_ext, bqkv_ext, wproj_ext, out_ext)
    nc.compile()
    return nc


def _body(nc, tc, x_ext, wqkv_ext, bqkv_ext, wproj_ext, out_ext):
    dma = nc.default_dma_engine

    from contextlib import ExitStack
    ctx = ExitStack()
    with ctx:
        singles = ctx.enter_context(tc.tile_pool(name="singles", bufs=1))
        qkt_pool = ctx.enter_context(tc.tile_pool(name="qkT", bufs=1))
        vpool = ctx.enter_context(tc.tile_pool(name="v", bufs=1))
        apool = ctx.enter_context(tc.tile_pool(name="attnT", bufs=1))
        ptpool = ctx.enter_context(tc.tile_pool(name="pt", bufs=5))
        rspool = ctx.enter_context(tc.tile_pool(name="rs", bufs=4))
        wp_pool = ctx.enter_context(tc.tile_pool(name="wp", bufs=1))
        opool = ctx.enter_context(tc.tile_pool(name="outs", bufs=4))
        psum = ctx.enter_context(tc.tile_pool(name="psum", bufs=1, space="PSUM"))
        xpool = ctx.enter_context(tc.tile_pool(name="x", bufs=1, side="right"))
        wv_pool = ctx.enter_context(tc.tile_pool(name="wv", bufs=1, side="right"))
        wqk_pool = ctx.enter_context(tc.tile_pool(name="wqk", bufs=1, side="right"))

        # ---- HAM warmup: dummy fp32 matmuls (4 cyc/row, ~1.7us each when
        # cold) bridge the initial DMA ramp so the PE clock is at 8/8 when
        # the first real matmuls issue. Plus a dummy EXP to pre-load the
        # ACT spline table set (~2.7us) while DMA streams in.
        warm = rspool.tile([P, 1024], F32, tag="rs", name="warm")
        nc.vector.memset(warm[:], 1.0)
        for i in range(6):
            wps = psum.tile([P, 512], F32, tag="mm", bufs=2, name=f"warm{i}")
            nc.tensor.matmul(wps[:], warm[:, 0:P], warm[:, 0:512])
        exp_dummy = singles.tile([P, 8], BF16)
        nc.scalar.activation(exp_dummy[:], warm[:, 0:8],
                             mybir.ActivationFunctionType.Exp)

        # ---- constants ----
        mask = singles.tile([P, P], BF16)       # upper-tri (t_q >= t_k) 0/1
        make_upper_triangular(nc, mask[:], val=1.0, diag=True)

        bqk_t = singles.tile([P, JQK // P], F32)  # per-partition q/k biases
        dma.dma_start(out=bqk_t[:], in_=bqkv_ext[:JQK].rearrange("(j p) -> p j", p=P))
        bv_b = singles.tile([P, JV], F32)       # v bias broadcast over partitions
        bv_src = bass.AP(tensor=bqkv_ext, offset=JQK, ap=[[0, P], [1, JV]])
        dma.dma_start(out=bv_b[:], in_=bv_src)

        # ---- input DMAs, in need-order (queue priority = emission order) --
        wv = []
        for ct in range(NCT):
            wt = wv_pool.tile([P, JV], BF16, tag=f"wv{ct}", name=f"wv{ct}")
            dma.dma_start(out=wt[:], in_=wqkv_ext[ct * P:(ct + 1) * P, JQK:])
            wv.append(wt)
        # x: per (ct, tcn) tiles of [128, 512]
        xts = [[None] * NTC for _ in range(NCT)]

        def load_x(tcn):
            for ct in range(NCT):
                t_ = xpool.tile([P, 512], BF16, tag=f"x{ct}_{tcn}",
                                name=f"x{ct}_{tcn}")
                dma.dma_start(
                    out=t_[:],
                    in_=x_ext[ct * P:(ct + 1) * P, tcn * 512:(tcn + 1) * 512])
                xts[ct][tcn] = t_

        load_x(0)
        # wqk: [128, 1024] per ct, pair-major columns; pair0 slice first
        wqk = []
        for ct in range(NCT):
            wt = wqk_pool.tile([P, JQK], BF16, tag=f"wqk{ct}", name=f"wqk{ct}")
            dma.dma_start(out=wt[:, 0:256],
                          in_=wqkv_ext[ct * P:(ct + 1) * P, 0:256])
            wqk.append(wt)
        for tcn in range(1, NTC):
            load_x(tcn)
        for ct in range(NCT):
            dma.dma_start(out=wqk[ct][:, 256:],
                          in_=wqkv_ext[ct * P:(ct + 1) * P, 256:JQK])
        wproj = []
        for jt in range(NPJ):
            wt = wp_pool.tile([P, C], BF16, tag=f"wp{jt}", name=f"wp{jt}")
            dma.dma_start(out=wt[:], in_=wproj_ext[jt * P:(jt + 1) * P, :])
            wproj.append(wt)

        # ---- v pass ----
        # v_sb[kt]: [128, 1024] = per head h: [ones(64) | V_h(64)] at col 128h;
        # ones make attn@V emit the softmax denominator on partitions 0-63
        vts = []

        def v_tile(kt):
            psv = psum.tile([P, JV], F32, tag="mm", bufs=2, name=f"psv{kt}")
            for ct in range(NCT):
                nc.tensor.matmul(
                    psv[:],
                    xts[ct][kt // 4][:, (kt % 4) * P:(kt % 4 + 1) * P],
                    wv[ct][:],
                    start=(ct == 0), stop=(ct == NCT - 1),
                )
            vt = vpool.tile([P, 2 * JV], BF16, tag=f"v{kt}", name=f"v{kt}")
            vt3 = vt[:].rearrange("p (h c) -> p h c", h=HC)
            nc.gpsimd.memset(vt3[:, :, 0:64], 1.0)
            nc.vector.tensor_add(
                vt3[:, :, 64:128],
                psv[:].rearrange("p (h c) -> p h c", h=HC),
                bv_b[:].rearrange("p (h c) -> p h c", h=HC),
            )
            vts.append(vt)

        for kt in range(4):
            v_tile(kt)

        # ---- qk pass: per (pair, q/k, tcn) accumulation groups ----
        # qkT[p][0][tcn] = q^T chunk, qkT[p][1][tcn] = k^T chunk, [128, 512]
        qkT = {}

        def qk_tiles(p_):
            qs = [[None] * NTC, [None] * NTC]
            qkT[p_] = qs
            for tcn in range(NTC):
                for w_ in range(2):   # 0=q, 1=k
                    tname = ("q", "k")[w_]
                    t_ = qkt_pool.tile([P, 512], BF16,
                                       tag=f"{tname}{p_ % 2}_{tcn}",
                                       name=f"{tname}T{p_}_{tcn}")
                    ps = psum.tile([P, 512], F32, tag="mm", bufs=2,
                                   name=f"psqk{p_}_{w_}_{tcn}")
                    cols = 256 * p_ + 128 * w_
                    for ct in range(NCT):
                        nc.tensor.matmul(
                            ps[:], wqk[ct][:, cols:cols + 128],
                            xts[ct][tcn][:],
                            start=(ct == 0), stop=(ct == NCT - 1),
                        )
                    nc.vector.tensor_scalar_add(
                        out=t_[:], in0=ps[:],
                        scalar1=bqk_t[:, 2 * p_ + w_:2 * p_ + w_ + 1])
                    qs[w_][tcn] = t_

        # attnT[p][tcn]: [128 (2 heads x 64 d), 512] bf16
        attnT = {p_: [None] * NTC for p_ in range(4)}

        def attention(p_, qc):
            qT = qkT[p_][0][qc]
            pso = [psum.tile([P, 512], F32, tag="o", bufs=2,
                             name=f"pso{p_}_{qc}_{i}")
                   for i in range(2)]
            nkt = 4 * (qc + 1)
            pending = None
            for kt in range(nkt):
                o = max(0, kt * P - qc * 512)
                kT = qkT[p_][1][kt // 4]
                kcol = (kt % 4) * P
                ss = psum.tile([P, 1024], F32, tag="s", bufs=2,
                               name=f"pss{p_}_{qc}_{kt}")
                for h in range(2):
                    lo, hi = h * 64, (h + 1) * 64
                    nc.tensor.matmul(
                        ss[:, 512 * h + o:512 * (h + 1)],
                        kT[lo:hi, kcol:kcol + P],
                        qT[lo:hi, o:512],
                    )
                pt = ptpool.tile([P, 1024], BF16, tag="pt",
                                 name=f"pt{p_}_{qc}_{kt}")
                ss3 = ss[:].rearrange("p (h w) -> p h w", h=2)
                pt3 = pt[:].rearrange("p (h w) -> p h w", h=2)
                nc.scalar.activation(
                    pt3[:, :, o:], ss3[:, :, o:],
                    mybir.ActivationFunctionType.Exp,
                )
                if kt >= 4 * qc:
                    # diagonal block: zero t_q < t_k (both heads)
                    mask_b = bass.AP(
                        tensor=mask[:].tensor, offset=mask[:].offset,
                        ap=[mask[:].ap[0], [0, 2], [1, P]])
                    nc.vector.tensor_mul(
                        pt3[:, :, o:o + P], pt3[:, :, o:o + P], mask_b)
                if pending is not None:
                    _emit_av(nc, vts, pso, p_, *pending, nkt)
                pending = (pt, o, kt)
            _emit_av(nc, vts, pso, p_, *pending, nkt)
            # normalize: pso rows 0:64 = row-sum, 64:128 = outT
            at = apool.tile([P, 512], BF16, tag=f"a{p_}_{qc}",
                            name=f"attnT{p_}_{qc}")
            attnT[p_][qc] = at
            rsb = rspool.tile([P, 1024], F32, tag="rs", name=f"rs{p_}_{qc}")
            # fast recip is lane-locked: compute at base 0 (frees the psum
            # fast), one combined DMA-shift to partitions 64-127
            nc.vector.reciprocal_approx_fast(rsb[0:64, 0:512], pso[0][0:64, :])
            nc.vector.reciprocal_approx_fast(rsb[0:64, 512:], pso[1][0:64, :])
            dma.dma_start(out=rsb[64:128, :], in_=rsb[0:64, :])
            for h in range(2):
                nc.vector.tensor_mul(
                    at[64 * h:64 * h + 64, :],
                    pso[h][64:128, :], rsb[64:128, 512 * h:512 * h + 512])

        def proj_chunk(tcn):
            for mt in range(NMT):
                psp = psum.tile([P, 512], F32, tag="mm", bufs=2,
                                name=f"psp{mt}_{tcn}")
                for jt in range(NPJ):
                    nc.tensor.matmul(
                        psp[:], wproj[jt][:, mt * P:(mt + 1) * P],
                        attnT[jt][tcn][:],
                        start=(jt == 0), stop=(jt == NPJ - 1),
                    )
                ot = opool.tile([P, 512], BF16, tag="ot", name=f"ot{mt}_{tcn}")
                nc.vector.tensor_scalar_add(out=ot[:], in0=psp[:], scalar1=0.0)
                dma.dma_start(
                    out=out_ext[mt * P:(mt + 1) * P,
                                tcn * 512:(tcn + 1) * 512],
                    in_=ot[:])

        # ---- main emission: attention as early as deps allow; fill work
        # (v remainder, next pair's qk, proj) emitted after = lower priority
        qk_tiles(0)
        for qc in range(NTC):
            attention(0, qc)
            # v tiles for the NEXT chunk, emitted after this chunk's blocks:
            # they fill PE slack during the ACT-bound exp chain without
            # outranking the attention pipeline in scheduler priority
            if qc < NTC - 1:
                for kt in range(4 * (qc + 1), 4 * (qc + 2)):
                    v_tile(kt)
        for p_ in (1, 2):
            qk_tiles(p_)
            for qc in range(NTC):
                attention(p_, qc)
        qk_tiles(3)
        # pair 3 runs q-chunks high-to-low; each chunk completes attnT[*][qc]
        # so its projection chunk is emitted (and runs) immediately after,
        # leaving only proj(qc=0) as the tail
        for qc in range(NTC - 1, -1, -1):
            attention(3, qc)
            proj_chunk(qc)


def _emit_av(nc, vts, pso, p_, pt, o, kt, nkt):
    """attn@V for one (pair, kt) block: [ones|V_h].T @ P~ accumulated."""
    for h in range(2):
        head = 2 * p_ + h
        vaug = vts[kt][:, 128 * head:128 * head + 128]
        nc.tensor.matmul(
            pso[h][:, o:], vaug, pt[:, 512 * h + o:512 * (h + 1)],
            start=(kt == 0), stop=(kt == nkt - 1),
        )


def shard_inputs(x, W_qkv, b_qkv, W_proj, b_proj):
    """Build the 8 per-core input maps (host-side sharding)."""
    x = np.asarray(x, np.float32)
    W_qkv = np.asarray(W_qkv, np.float32)
    b_qkv = np.asarray(b_qkv, np.float32)
    W_proj = np.asarray(W_proj, np.float32)
    in_maps = []
    for c in range(NCORES):
        b, g = c // 2, c % 2
        s = slice(512 * g, 512 * g + 512)
        Wq = W_qkv[0 * C:1 * C][s] * 0.125
        Wk = W_qkv[1 * C:2 * C][s]
        Wv = W_qkv[2 * C:3 * C][s]
        bq = b_qkv[0 * C:1 * C][s] * 0.125
        bk = b_qkv[1 * C:2 * C][s]
        # pair-major interleave: [q0|k0|q1|k1|q2|k2|q3|k3|v]
        qk_rows = []
        bqk = []
        for p_ in range(4):
            qk_rows.append(Wq[128 * p_:128 * (p_ + 1)])
            qk_rows.append(Wk[128 * p_:128 * (p_ + 1)])
            bqk.append(bq[128 * p_:128 * (p_ + 1)])
            bqk.append(bk[128 * p_:128 * (p_ + 1)])
        wqkv = np.ascontiguousarray(
            np.concatenate(qk_rows + [Wv], 0).T).astype(ml_dtypes.bfloat16)
        bv = b_qkv[2 * C:3 * C][s]
        in_maps.append({
            "xT": np.ascontiguousarray(x[b].T).astype(ml_dtypes.bfloat16),
            "wqkv": wqkv,
            "bqkv": np.ascontiguousarray(np.concatenate(bqk + [bv])),
            "wproj": np.ascontiguousarray(W_proj[:, s].T).astype(ml_dtypes.bfloat16),
        })
    return in_maps


def run(in_maps, trace=False):
    global _compiled
    if _compiled is None:
        _compiled = build()
    return run_bass_kernel_spmd(
        _compiled, in_maps, core_ids=list(range(NCORES)), trace=trace)


def postprocess(res, b_proj):
    b_proj = np.asarray(b_proj, np.float32)
    out = np.empty((B, T, C), np.float32)
    for b in range(B):
        partial = (res.results[2 * b]["out"].astype(np.float32)
                   + res.results[2 * b + 1]["out"].astype(np.float32))
        out[b] = partial.T + b_proj
    return out


def kernel(x, W_qkv, b_qkv, W_proj, b_proj):
    in_maps = shard_inputs(x, W_qkv, b_qkv, W_proj, b_proj)
    res = run(in_maps)
    return postprocess(res, b_proj)


if __name__ == "__main__":
    rng = np.random.default_rng(0)
    xs = {
        "x": rng.standard_normal((B, T, C)).astype(np.float32),
        "W_qkv": (rng.standard_normal((3 * C, C)) / 32).astype(np.float32),
        "b_qkv": (rng.standard_normal(3 * C) * 0.02).astype(np.float32),
        "W_proj": (rng.standard_normal((C, C)) / 32).astype(np.float32),
        "b_proj": (rng.standard_normal(C) * 0.02).astype(np.float32),
    }
    out = kernel(**xs)
    print("out", out.shape, out.dtype, np.abs(out).mean())
